# revision 9
# baseline (speedup 1.0000x reference)
"""Softmax-weighted nearest-neighbor aggregation (DiffusionStar) on 8 TRN2 cores.

Strategy (v4):
  - Shard the train set (N=50000) across 8 cores (6250 rows each, padded to 6272).
  - Two-phase softmax per core. DMA-bytes minimization vs the 76.8MB/core
    baseline (which streamed the train slice twice, once per layout):
      * transposed copy (tT) stored group-contiguous in DRAM (24KB/partition
        descriptors),
      * -||t||^2 folded into the score GEMM as a K=2 fp16 hi/lo matmul,
      * 14 of 49 natural-layout chunks are never re-streamed: their kT tiles
        are transposed on-chip (5 chunks via PE identity-transpose + ACT
        copies, 5 via DVE StreamTranspose 32x32 block ops, 4 more JIT'd on DVE
        at the start of phase 2 from the last two resident kT groups).
  - Scheduling: transpose work drains from FIFO queues at a bounded rate per
    score-group slot (<=12 PE tiles / <=8 DVE instrs) so the kT double-buffer
    rotation never stalls the tT DMA stream; natural-chunk loads go on the ACT
    HWDGE ring (independent FIFO from the tT loads on the sync ring) so they
    prefetch from t=0 under natp-pool backpressure; cached chunks' GEMM2
    matmuls are deferred to the end so the tail never waits on DMA.
  - Host merges (M, S, ACC) across cores with the online-softmax combine in
    fp64.

Numerics identical to the validated baseline (rel err ~5e-4): fp16 GEMMs,
fp32 PSUM, exact hi/lo fp16 trsq, pad columns excluded exactly via the
106-valid-col last group.
"""

import numpy as np

B = 64
D = 3072
N = 50000
NCORES = 8
N_LOC = N // NCORES          # 6250
N_PAD = 6272                 # 49 * 128
KD = D // 128                # 24
KN = N_PAD // 128            # 49
DJ = D // 512                # 6
GROUPS = [(i * 512, 512) for i in range(12)] + [(6144, 128)]
NG = len(GROUPS)
W_LAST = N_LOC - 6144        # 106 valid columns in the last group
NAT_BUFS = 4

# on-chip-transposed chunks (never re-streamed in natural layout)
CACHED_P1_PE = [4 * g + 1 for g in (1, 3, 5, 7, 9)]      # PE path, phase 1
CACHED_P1_DVE = [4 * g + 1 for g in (2, 4, 6, 8, 10)]    # DVE path, phase 1
CACHED_P2_DVE = [44, 45, 46, 48]                          # DVE JIT, phase 2
CACHED = set(CACHED_P1_PE) | set(CACHED_P1_DVE) | set(CACHED_P2_DVE)
PE_DRAIN_PER_SLOT = 12
DVE_DRAIN_PER_SLOT = 8

_CACHED = {}


def _build_nc():
    import concourse.bacc as bacc
    import concourse.tile as tile
    from concourse import mybir
    from contextlib import ExitStack

    f16 = mybir.dt.float16
    f32 = mybir.dt.float32

    nc = bacc.Bacc("TRN2", target_bir_lowering=False, debug=False)

    tTg = nc.dram_tensor("tTg", [12 * 128, KD * 512], f16, kind="ExternalInput").ap()
    tTl = nc.dram_tensor("tTl", [128, KD * 128], f16, kind="ExternalInput").ap()
    tn = nc.dram_tensor("tn", [N_PAD, D], f16, kind="ExternalInput").ap()
    xT = nc.dram_tensor("xT", [D, B], f16, kind="ExternalInput").ap()
    ident = nc.dram_tensor("ident", [128, 128], f16, kind="ExternalInput").ap()
    # hi/lo fp16 split of ||t||^2: rows (32m, 32m+1) hold group (3s+m)'s
    # (hi, lo) so the K=2 matmul rhs has a legal base partition
    trsq = nc.dram_tensor("trsq", [96, 5 * 512], f16, kind="ExternalInput").ap()
    negs = nc.dram_tensor("negs", [96, B], f16, kind="ExternalInput").ap()
    gcol = nc.dram_tensor("gcol", [B, 1], f32, kind="ExternalInput").ap()

    acc_out = nc.dram_tensor("acc_out", [B, D], f32, kind="ExternalOutput").ap()
    s_out = nc.dram_tensor("s_out", [B, 1], f32, kind="ExternalOutput").ap()
    m_out = nc.dram_tensor("m_out", [B, 1], f32, kind="ExternalOutput").ap()

    with tile.TileContext(nc) as tc, ExitStack() as ctx:
        const = ctx.enter_context(tc.tile_pool(name="const", bufs=1))
        kTp = ctx.enter_context(tc.tile_pool(name="kT", bufs=2))
        cachep = ctx.enter_context(tc.tile_pool(name="cache", bufs=1))
        natp = ctx.enter_context(tc.tile_pool(name="nat", bufs=NAT_BUFS))
        sb = ctx.enter_context(tc.tile_pool(name="sb", bufs=1))
        pTp = ctx.enter_context(tc.tile_pool(name="pTp", bufs=4))
        pTs = ctx.enter_context(tc.tile_pool(name="pTs", bufs=1))
        pp = ctx.enter_context(tc.tile_pool(name="pp", bufs=2))
        accst = ctx.enter_context(tc.tile_pool(name="accst", bufs=2))

        # --- constants ---
        xT_sb = const.tile([128, KD, B], f16)
        nc.sync.dma_start(xT_sb[:], xT.rearrange("(k p) b -> p k b", p=128))
        id_sb = const.tile([128, 128], f16)
        nc.sync.dma_start(id_sb[:], ident[:])
        trsq_sb = const.tile([96, 5, 512], f16)
        nc.sync.dma_start(trsq_sb[:], trsq.rearrange("p (s n) -> p s n", n=512))
        neg_sb = const.tile([96, B], f16)
        nc.sync.dma_start(neg_sb[:], negs[:])
        g_sb = const.tile([B, 1], f32)
        nc.sync.dma_start(g_sb[:], gcol[:])

        mpart = sb.tile([B, NG], f32)
        ssum = sb.tile([B, NG], f32)
        stat = sb.tile([B, 4], f32)
        sc_tiles = []
        p_tiles = {}
        pT_stash = {}
        nat_cache = {}
        kT_tiles = {}

        def alloc_nat(c):
            nat = cachep.tile([128, D], f16, tag=f"natc{c}")
            nat_cache[c] = nat
            return nat

        # DVE-path work item: one (i, j) StreamTranspose covering all 24 k
        def dve_item(kT_t, ci, nat, i, j):
            natv = nat.rearrange("p (k d) -> p k d", d=128)
            nc.vector.transpose(
                natv[32 * j:32 * j + 32, :, 32 * i:32 * i + 32],
                kT_t[32 * i:32 * i + 32, :,
                     ci * 128 + 32 * j:ci * 128 + 32 * j + 32])

        # ---------------- phase 1 ----------------
        work_pe = []
        work_dve = []
        with tc.tile_pool(name="psS", bufs=2, space="PSUM") as psS, \
             tc.tile_pool(name="psT", bufs=4, space="PSUM") as psT:

            # PE-path work item: one 128x128 tile (transpose + ACT copy)
            def pe_item(kT_t, ci, nat, k):
                pt = psT.tile([128, 128], f16, tag="pt")
                nc.tensor.transpose(pt[:], kT_t[:, k, ci * 128:(ci + 1) * 128],
                                    id_sb[:])
                nc.scalar.copy(nat[:, k * 128:(k + 1) * 128], pt[:])

            for gi, (n0, W) in enumerate(GROUPS):
                kT = kTp.tile([128, KD, 512], f16, tag="kT")
                kT_tiles[gi] = kT
                if gi < 12:
                    nc.sync.dma_start(
                        kT[:, :, :W],
                        tTg[gi * 128:(gi + 1) * 128, :].rearrange(
                            "p (k n) -> p k n", n=512))
                else:
                    nc.sync.dma_start(
                        kT[:, :, :W],
                        tTl[:, :].rearrange("p (k n) -> p k n", n=128))

                ps = psS.tile([B, 512], f32, tag="ps")
                bp = 32 * (gi % 3)
                nc.tensor.matmul(ps[:, :W], neg_sb[bp:bp + 2, :],
                                 trsq_sb[bp:bp + 2, gi // 3, :W],
                                 start=True, stop=False)
                for k in range(KD):
                    nc.tensor.matmul(ps[:, :W], xT_sb[:, k, :], kT[:, k, :W],
                                     start=False, stop=(k == KD - 1))
                WE = W if gi < 12 else W_LAST
                sc = sb.tile([B, 512], f32, tag=f"sc{gi}")
                sc_tiles.append(sc)
                nc.scalar.copy(sc[:, :W], ps[:, :W])
                nc.vector.reduce_max(mpart[:, gi:gi + 1], sc[:, :WE],
                                     axis=mybir.AxisListType.X)

                # enqueue this group's cached-chunk transpose items
                for ci in range(W // 128):
                    c = 4 * gi + ci
                    if c in CACHED_P1_PE:
                        nat = alloc_nat(c)
                        for k in range(KD):
                            work_pe.append((kT, ci, nat, k))
                    elif c in CACHED_P1_DVE:
                        nat = alloc_nat(c)
                        for i in range(4):
                            for j in range(4):
                                work_dve.append((kT, ci, nat, i, j))

                # drain bounded amounts so the kT rotation never stalls
                for _ in range(min(PE_DRAIN_PER_SLOT, len(work_pe))):
                    pe_item(*work_pe.pop(0))
                for _ in range(min(DVE_DRAIN_PER_SLOT, len(work_dve))):
                    dve_item(*work_dve.pop(0))

            # flush leftovers (should be nearly empty)
            while work_pe:
                pe_item(*work_pe.pop(0))
            while work_dve:
                dve_item(*work_dve.pop(0))

        # --- global max, bias = -g*M ---
        nc.vector.reduce_max(stat[:, 0:1], mpart[:, :NG],
                             axis=mybir.AxisListType.X)
        nc.vector.tensor_tensor(stat[:, 2:3], g_sb[:], stat[:, 0:1],
                                op=mybir.AluOpType.mult)
        nc.vector.tensor_scalar_mul(stat[:, 2:3], stat[:, 2:3], -1.0)

        # phase-2 JIT transposes (group 11/12 kT tiles are still resident)
        jit_items = []
        for c in CACHED_P2_DVE:
            nat = alloc_nat(c)
            for i in range(4):
                for j in range(4):
                    jit_items.append((kT_tiles[c // 4], c % 4, nat, i, j))

        # ---------------- phase 2: exp -> pT -> GEMM2 ----------------
        with tc.tile_pool(name="psT2", bufs=2, space="PSUM") as psT2, \
             tc.tile_pool(name="psA", bufs=1, space="PSUM") as psA:
            acc_ps = psA.tile([B, DJ, 512], f32)

            first = True
            for c in range(KN):
                gi = c // 4
                ci = c % 4
                n0, W = GROUPS[gi]
                if ci == 0:
                    WE = W if gi < 12 else W_LAST
                    p = pp.tile([B, 512], f16, tag="p")
                    p_tiles[gi] = p
                    if WE < W:
                        nc.vector.memset(p[:, WE:W], 0.0)
                    nc.scalar.activation(p[:, :WE], sc_tiles[gi][:, :WE],
                                         mybir.ActivationFunctionType.Exp,
                                         bias=stat[:, 2:3], scale=g_sb[:],
                                         accum_out=ssum[:, gi:gi + 1])
                pt2 = psT2.tile([128, B], f16, tag="pt2")
                nc.tensor.transpose(pt2[:],
                                    p_tiles[gi][:, ci * 128:(ci + 1) * 128],
                                    id_sb[:B, :B])
                if c in CACHED:
                    pT = pTs.tile([128, B], f16, tag=f"pTs{c}")
                    pT_stash[c] = pT
                else:
                    pT = pTp.tile([128, B], f16, tag="pT")
                nc.vector.tensor_copy(pT[:], pt2[:])

                if c not in CACHED:
                    nat = natp.tile([128, D], f16, tag="nat")
                    # ACT HWDGE ring: independent FIFO from the sync-ring tT
                    # loads, so these prefetch from t=0 under pool backpressure
                    nc.scalar.dma_start(nat[:], tn[c * 128:(c + 1) * 128, :])
                    for j in range(DJ):
                        nc.tensor.matmul(acc_ps[:, j, :], pT[:],
                                         nat[:, j * 512:(j + 1) * 512],
                                         start=first, stop=False)
                    first = False

                # spread the JIT stream-transposes across the DVE timeline
                for _ in range(2):
                    if jit_items:
                        dve_item(*jit_items.pop(0))

            while jit_items:
                dve_item(*jit_items.pop(0))

            # deferred GEMM2 for cached chunks (everything resident in SBUF)
            cached_sorted = sorted(CACHED)
            for idx, c in enumerate(cached_sorted):
                last = idx == len(cached_sorted) - 1
                for j in range(DJ):
                    nc.tensor.matmul(acc_ps[:, j, :], pT_stash[c][:],
                                     nat_cache[c][:, j * 512:(j + 1) * 512],
                                     start=False, stop=last)

            for j in range(DJ):
                st = accst.tile([B, 512], f32, tag="accst")
                nc.scalar.copy(st[:], acc_ps[:, j, :])
                nc.sync.dma_start(acc_out[:, j * 512:(j + 1) * 512], st[:])

        nc.vector.reduce_sum(stat[:, 1:2], ssum[:, :NG],
                             axis=mybir.AxisListType.X)
        nc.sync.dma_start(s_out[:], stat[:, 1:2])
        nc.sync.dma_start(m_out[:], stat[:, 0:1])

    nc.compile()
    return nc


def _get_nc():
    if "nc" not in _CACHED:
        _CACHED["nc"] = _build_nc()
    return _CACHED["nc"]


def kernel(x, train, alphas_cumprod, t, **_unused):
    from concourse.bass_utils import run_bass_kernel_spmd

    x = np.asarray(x)
    train = np.asarray(train)
    alphas_cumprod = np.asarray(alphas_cumprod)
    t = np.asarray(t).astype(np.int64)

    xf = x.reshape(B, -1).astype(np.float32)
    tf = train.reshape(N, -1).astype(np.float32)

    acp_t = alphas_cumprod.astype(np.float64)[t]
    a = np.sqrt(acp_t)
    om = 1.0 - acp_t
    gp32 = (a * a / (2.0 * om)).astype(np.float32)   # softmax scale on s''
    xscale = (2.0 / a).astype(np.float32)            # fold into x

    trsq_full = np.einsum("nd,nd->n", tf.astype(np.float64),
                          tf.astype(np.float64)).astype(np.float32)

    t16 = tf.astype(np.float16)
    x16T = np.ascontiguousarray(
        (xscale[:, None] * xf).astype(np.float16).T)  # [D, B]
    ident = np.eye(128, dtype=np.float16)
    g_col = gp32.reshape(B, 1)
    negs = np.zeros((96, B), dtype=np.float16)
    negs[[0, 1, 32, 33, 64, 65], :] = -1.0

    in_maps = []
    for c in range(NCORES):
        sl = slice(c * N_LOC, (c + 1) * N_LOC)
        nat = np.zeros((N_PAD, D), dtype=np.float16)
        nat[:N_LOC] = t16[sl]
        # group-contiguous transposed layout: per group g, [128 d-part, 24 k, W n]
        tTg = np.empty((12, 128, KD, 512), dtype=np.float16)
        for g in range(12):
            blk = nat[g * 512:(g + 1) * 512, :]          # [512 n, 3072 d]
            tTg[g] = blk.reshape(512, KD, 128).transpose(2, 1, 0)
        tTl = nat[6144:6272, :].reshape(128, KD, 128).transpose(2, 1, 0)
        trsq_c = np.zeros((N_PAD,), dtype=np.float32)
        trsq_c[:N_LOC] = trsq_full[sl]
        hi = trsq_c.astype(np.float16)
        lo = (trsq_c.astype(np.float64) - hi.astype(np.float64)).astype(np.float16)
        trsqg = np.zeros((96, 5, 512), dtype=np.float16)
        for g in range(NG):
            w = GROUPS[g][1]
            trsqg[32 * (g % 3), g // 3, :w] = hi[g * 512:g * 512 + w]
            trsqg[32 * (g % 3) + 1, g // 3, :w] = lo[g * 512:g * 512 + w]
        in_maps.append({
            "tTg": np.ascontiguousarray(tTg.reshape(12 * 128, KD * 512)),
            "tTl": np.ascontiguousarray(tTl.reshape(128, KD * 128)),
            "tn": nat,
            "xT": x16T,
            "ident": ident,
            "trsq": trsqg.reshape(96, 5 * 512),
            "negs": negs,
            "gcol": g_col,
        })

    nc = _get_nc()
    res = run_bass_kernel_spmd(nc, in_maps, list(range(NCORES)))
    _CACHED["last_results"] = res

    # --- host-side online-softmax merge across cores (fp64) ---
    g64 = gp32.astype(np.float64)
    Ms = np.stack([res.results[c]["m_out"][:, 0].astype(np.float64)
                   for c in range(NCORES)])          # [C, B]
    Ss = np.stack([res.results[c]["s_out"][:, 0].astype(np.float64)
                   for c in range(NCORES)])          # [C, B]
    ACCs = np.stack([res.results[c]["acc_out"].astype(np.float64)
                     for c in range(NCORES)])        # [C, B, D]
    Mg = Ms.max(axis=0)                              # [B]
    scale = np.exp(g64[None, :] * (Ms - Mg[None, :]))  # [C, B]
    den = (scale * Ss).sum(axis=0)                   # [B]
    num = (scale[:, :, None] * ACCs).sum(axis=0)     # [B, D]
    weighted = num / den[:, None]

    coef_x = 1.0 / np.sqrt(om)
    coef_x_hat = a / np.sqrt(om)
    out = coef_x[:, None] * xf.astype(np.float64) - coef_x_hat[:, None] * weighted
    return out.reshape(x.shape).astype(np.float32)


# revision 10
# speedup vs baseline: 1.0249x; 1.0249x over previous
"""Softmax-weighted nearest-neighbor aggregation (DiffusionStar) on 8 TRN2 cores.

Strategy (v4):
  - Shard the train set (N=50000) across 8 cores (6250 rows each, padded to 6272).
  - Two-phase softmax per core. DMA-bytes minimization vs the 76.8MB/core
    baseline (which streamed the train slice twice, once per layout):
      * transposed copy (tT) stored group-contiguous in DRAM (24KB/partition
        descriptors),
      * -||t||^2 folded into the score GEMM as a K=2 fp16 hi/lo matmul,
      * 14 of 49 natural-layout chunks are never re-streamed: their kT tiles
        are transposed on-chip (5 chunks via PE identity-transpose + ACT
        copies, 5 via DVE StreamTranspose 32x32 block ops, 4 more JIT'd on DVE
        at the start of phase 2 from the last two resident kT groups).
  - Scheduling: transpose work drains from FIFO queues at a bounded rate per
    score-group slot (<=12 PE tiles / <=8 DVE instrs) so the kT double-buffer
    rotation never stalls the tT DMA stream; natural-chunk loads go on the ACT
    HWDGE ring (independent FIFO from the tT loads on the sync ring) so they
    prefetch from t=0 under natp-pool backpressure; cached chunks' GEMM2
    matmuls are deferred to the end so the tail never waits on DMA.
  - Host merges (M, S, ACC) across cores with the online-softmax combine in
    fp64.

Numerics identical to the validated baseline (rel err ~5e-4): fp16 GEMMs,
fp32 PSUM, exact hi/lo fp16 trsq, pad columns excluded exactly via the
106-valid-col last group.
"""

import numpy as np

B = 64
D = 3072
N = 50000
NCORES = 8
N_LOC = N // NCORES          # 6250
N_PAD = 6272                 # 49 * 128
KD = D // 128                # 24
KN = N_PAD // 128            # 49
DJ = D // 512                # 6
GROUPS = [(i * 512, 512) for i in range(12)] + [(6144, 128)]
NG = len(GROUPS)
W_LAST = N_LOC - 6144        # 106 valid columns in the last group
NAT_BUFS = 4

# on-chip-transposed chunks (never re-streamed in natural layout)
CACHED_P1_PE = [4 * g + 1 for g in (1, 3, 5, 7, 9)]      # PE path, phase 1
CACHED_P1_DVE = [4 * g + 1 for g in (2, 4, 6, 8, 10)]    # DVE path, phase 1
CACHED_P2_DVE = [44, 45, 46, 48]                          # DVE JIT, phase 2
CACHED = set(CACHED_P1_PE) | set(CACHED_P1_DVE) | set(CACHED_P2_DVE)
PE_DRAIN_PER_SLOT = 12
DVE_DRAIN_PER_SLOT = 8

_CACHED = {}


def _build_nc():
    import concourse.bacc as bacc
    import concourse.tile as tile
    from concourse import mybir
    from contextlib import ExitStack

    f16 = mybir.dt.float16
    f32 = mybir.dt.float32

    nc = bacc.Bacc("TRN2", target_bir_lowering=False, debug=False)

    tTg = nc.dram_tensor("tTg", [12 * 128, KD * 512], f16, kind="ExternalInput").ap()
    tTl = nc.dram_tensor("tTl", [128, KD * 128], f16, kind="ExternalInput").ap()
    tn = nc.dram_tensor("tn", [N_PAD, D], f16, kind="ExternalInput").ap()
    xT = nc.dram_tensor("xT", [D, B], f16, kind="ExternalInput").ap()
    ident = nc.dram_tensor("ident", [128, 128], f16, kind="ExternalInput").ap()
    # hi/lo fp16 split of ||t||^2: rows (32m, 32m+1) hold group (3s+m)'s
    # (hi, lo) so the K=2 matmul rhs has a legal base partition
    trsq = nc.dram_tensor("trsq", [96, 5 * 512], f16, kind="ExternalInput").ap()
    negs = nc.dram_tensor("negs", [96, B], f16, kind="ExternalInput").ap()
    gcol = nc.dram_tensor("gcol", [B, 1], f32, kind="ExternalInput").ap()

    acc_out = nc.dram_tensor("acc_out", [B, D], f32, kind="ExternalOutput").ap()
    s_out = nc.dram_tensor("s_out", [B, 1], f32, kind="ExternalOutput").ap()
    m_out = nc.dram_tensor("m_out", [B, 1], f32, kind="ExternalOutput").ap()

    with tile.TileContext(nc) as tc, ExitStack() as ctx:
        const = ctx.enter_context(tc.tile_pool(name="const", bufs=1))
        kTp = ctx.enter_context(tc.tile_pool(name="kT", bufs=2))
        cachep = ctx.enter_context(tc.tile_pool(name="cache", bufs=1))
        natp = ctx.enter_context(tc.tile_pool(name="nat", bufs=NAT_BUFS))
        sb = ctx.enter_context(tc.tile_pool(name="sb", bufs=1))
        pTp = ctx.enter_context(tc.tile_pool(name="pTp", bufs=4))
        pTs = ctx.enter_context(tc.tile_pool(name="pTs", bufs=1))
        pp = ctx.enter_context(tc.tile_pool(name="pp", bufs=2))
        accst = ctx.enter_context(tc.tile_pool(name="accst", bufs=2))

        # --- constants ---
        xT_sb = const.tile([128, KD, B], f16)
        nc.sync.dma_start(xT_sb[:], xT.rearrange("(k p) b -> p k b", p=128))
        id_sb = const.tile([128, 128], f16)
        nc.sync.dma_start(id_sb[:], ident[:])
        trsq_sb = const.tile([96, 5, 512], f16)
        nc.sync.dma_start(trsq_sb[:], trsq.rearrange("p (s n) -> p s n", n=512))
        neg_sb = const.tile([96, B], f16)
        nc.sync.dma_start(neg_sb[:], negs[:])
        g_sb = const.tile([B, 1], f32)
        nc.sync.dma_start(g_sb[:], gcol[:])

        mpart = sb.tile([B, NG], f32)
        ssum = sb.tile([B, NG], f32)
        stat = sb.tile([B, 4], f32)
        sc_tiles = []
        p_tiles = {}
        pT_stash = {}
        nat_cache = {}
        kT_tiles = {}

        def alloc_nat(c):
            nat = cachep.tile([128, D], f16, tag=f"natc{c}")
            nat_cache[c] = nat
            return nat

        # DVE-path work item: one (i, j) StreamTranspose covering all 24 k
        def dve_item(kT_t, ci, nat, i, j):
            natv = nat.rearrange("p (k d) -> p k d", d=128)
            nc.vector.transpose(
                natv[32 * j:32 * j + 32, :, 32 * i:32 * i + 32],
                kT_t[32 * i:32 * i + 32, :,
                     ci * 128 + 32 * j:ci * 128 + 32 * j + 32])

        # ---------------- phase 1 ----------------
        work_pe = []
        work_dve = []
        with tc.tile_pool(name="psS", bufs=2, space="PSUM") as psS, \
             tc.tile_pool(name="psT", bufs=4, space="PSUM") as psT:

            # PE-path work item: one 128x128 tile (transpose + ACT copy)
            def pe_item(kT_t, ci, nat, k):
                pt = psT.tile([128, 128], f16, tag="pt")
                nc.tensor.transpose(pt[:], kT_t[:, k, ci * 128:(ci + 1) * 128],
                                    id_sb[:])
                nc.scalar.copy(nat[:, k * 128:(k + 1) * 128], pt[:])

            for gi, (n0, W) in enumerate(GROUPS):
                kT = kTp.tile([128, KD, 512], f16, tag="kT")
                kT_tiles[gi] = kT
                if gi < 12:
                    nc.sync.dma_start(
                        kT[:, :, :W],
                        tTg[gi * 128:(gi + 1) * 128, :].rearrange(
                            "p (k n) -> p k n", n=512))
                else:
                    nc.sync.dma_start(
                        kT[:, :, :W],
                        tTl[:, :].rearrange("p (k n) -> p k n", n=128))

                ps = psS.tile([B, 512], f32, tag="ps")
                bp = 32 * (gi % 3)
                nc.tensor.matmul(ps[:, :W], neg_sb[bp:bp + 2, :],
                                 trsq_sb[bp:bp + 2, gi // 3, :W],
                                 start=True, stop=False)
                for k in range(KD):
                    nc.tensor.matmul(ps[:, :W], xT_sb[:, k, :], kT[:, k, :W],
                                     start=False, stop=(k == KD - 1))
                WE = W if gi < 12 else W_LAST
                sc = sb.tile([B, 512], f32, tag=f"sc{gi}")
                sc_tiles.append(sc)
                nc.scalar.copy(sc[:, :W], ps[:, :W])
                nc.vector.reduce_max(mpart[:, gi:gi + 1], sc[:, :WE],
                                     axis=mybir.AxisListType.X)

                # enqueue this group's cached-chunk transpose items
                for ci in range(W // 128):
                    c = 4 * gi + ci
                    if c in CACHED_P1_PE:
                        nat = alloc_nat(c)
                        for k in range(KD):
                            work_pe.append((kT, ci, nat, k))
                    elif c in CACHED_P1_DVE:
                        nat = alloc_nat(c)
                        for i in range(4):
                            for j in range(4):
                                work_dve.append((kT, ci, nat, i, j))

                # drain bounded amounts so the kT rotation never stalls
                for _ in range(min(PE_DRAIN_PER_SLOT, len(work_pe))):
                    pe_item(*work_pe.pop(0))
                for _ in range(min(DVE_DRAIN_PER_SLOT, len(work_dve))):
                    dve_item(*work_dve.pop(0))

            # flush leftovers (should be nearly empty)
            while work_pe:
                pe_item(*work_pe.pop(0))
            while work_dve:
                dve_item(*work_dve.pop(0))

        # --- global max, bias = -g*M ---
        nc.vector.reduce_max(stat[:, 0:1], mpart[:, :NG],
                             axis=mybir.AxisListType.X)
        nc.vector.tensor_tensor(stat[:, 2:3], g_sb[:], stat[:, 0:1],
                                op=mybir.AluOpType.mult)
        nc.vector.tensor_scalar_mul(stat[:, 2:3], stat[:, 2:3], -1.0)

        # phase-2 JIT transposes (group 11/12 kT tiles are still resident)
        jit_items = []
        for c in CACHED_P2_DVE:
            nat = alloc_nat(c)
            for i in range(4):
                for j in range(4):
                    jit_items.append((kT_tiles[c // 4], c % 4, nat, i, j))

        # ---------------- phase 2: exp -> pT -> GEMM2 ----------------
        with tc.tile_pool(name="psT2", bufs=2, space="PSUM") as psT2, \
             tc.tile_pool(name="psA", bufs=1, space="PSUM") as psA:
            acc_ps = psA.tile([B, DJ, 512], f32)

            deferred = set(CACHED)
            first = True
            for c in range(KN):
                gi = c // 4
                ci = c % 4
                n0, W = GROUPS[gi]
                if ci == 0:
                    WE = W if gi < 12 else W_LAST
                    p = pp.tile([B, 512], f16, tag="p")
                    p_tiles[gi] = p
                    if WE < W:
                        nc.vector.memset(p[:, WE:W], 0.0)
                    nc.scalar.activation(p[:, :WE], sc_tiles[gi][:, :WE],
                                         mybir.ActivationFunctionType.Exp,
                                         bias=stat[:, 2:3], scale=g_sb[:],
                                         accum_out=ssum[:, gi:gi + 1])
                pt2 = psT2.tile([128, B], f16, tag="pt2")
                nc.tensor.transpose(pt2[:],
                                    p_tiles[gi][:, ci * 128:(ci + 1) * 128],
                                    id_sb[:B, :B])
                if c in CACHED:
                    pT = pTs.tile([128, B], f16, tag=f"pTs{c}")
                    pT_stash[c] = pT
                else:
                    pT = pTp.tile([128, B], f16, tag="pT")
                nc.scalar.copy(pT[:], pt2[:])

                if c not in CACHED:
                    nat = natp.tile([128, D], f16, tag="nat")
                    # GpSimd SWDGE ring: independent of the sync-ring tT loads
                    # and of ACT/DVE compute queues; prefetches from t=0 under
                    # pool backpressure
                    nc.gpsimd.dma_start(nat[:], tn[c * 128:(c + 1) * 128, :])
                    for j in range(DJ):
                        nc.tensor.matmul(acc_ps[:, j, :], pT[:],
                                         nat[:, j * 512:(j + 1) * 512],
                                         start=first, stop=False)
                    first = False

                # spread the JIT stream-transposes across the DVE timeline
                for _ in range(2):
                    if jit_items:
                        dve_item(*jit_items.pop(0))

                # fill PE idle slots with deferred cached-chunk GEMM2s
                if c % 3 == 2:
                    for cc in sorted(deferred):
                        if cc < c and cc not in CACHED_P2_DVE:
                            deferred.remove(cc)
                            for j in range(DJ):
                                nc.tensor.matmul(
                                    acc_ps[:, j, :], pT_stash[cc][:],
                                    nat_cache[cc][:, j * 512:(j + 1) * 512],
                                    start=False, stop=False)
                            break

            while jit_items:
                dve_item(*jit_items.pop(0))

            # remaining deferred GEMM2s (everything resident in SBUF)
            rem = sorted(deferred)
            for idx, c in enumerate(rem):
                last = idx == len(rem) - 1
                for j in range(DJ):
                    nc.tensor.matmul(acc_ps[:, j, :], pT_stash[c][:],
                                     nat_cache[c][:, j * 512:(j + 1) * 512],
                                     start=False, stop=last)

            for j in range(DJ):
                st = accst.tile([B, 512], f32, tag="accst")
                nc.scalar.copy(st[:], acc_ps[:, j, :])
                nc.sync.dma_start(acc_out[:, j * 512:(j + 1) * 512], st[:])

        nc.vector.reduce_sum(stat[:, 1:2], ssum[:, :NG],
                             axis=mybir.AxisListType.X)
        nc.sync.dma_start(s_out[:], stat[:, 1:2])
        nc.sync.dma_start(m_out[:], stat[:, 0:1])

    nc.compile()
    return nc


def _get_nc():
    if "nc" not in _CACHED:
        _CACHED["nc"] = _build_nc()
    return _CACHED["nc"]


def kernel(x, train, alphas_cumprod, t, **_unused):
    from concourse.bass_utils import run_bass_kernel_spmd

    x = np.asarray(x)
    train = np.asarray(train)
    alphas_cumprod = np.asarray(alphas_cumprod)
    t = np.asarray(t).astype(np.int64)

    xf = x.reshape(B, -1).astype(np.float32)
    tf = train.reshape(N, -1).astype(np.float32)

    acp_t = alphas_cumprod.astype(np.float64)[t]
    a = np.sqrt(acp_t)
    om = 1.0 - acp_t
    gp32 = (a * a / (2.0 * om)).astype(np.float32)   # softmax scale on s''
    xscale = (2.0 / a).astype(np.float32)            # fold into x

    trsq_full = np.einsum("nd,nd->n", tf.astype(np.float64),
                          tf.astype(np.float64)).astype(np.float32)

    t16 = tf.astype(np.float16)
    x16T = np.ascontiguousarray(
        (xscale[:, None] * xf).astype(np.float16).T)  # [D, B]
    ident = np.eye(128, dtype=np.float16)
    g_col = gp32.reshape(B, 1)
    negs = np.zeros((96, B), dtype=np.float16)
    negs[[0, 1, 32, 33, 64, 65], :] = -1.0

    in_maps = []
    for c in range(NCORES):
        sl = slice(c * N_LOC, (c + 1) * N_LOC)
        nat = np.zeros((N_PAD, D), dtype=np.float16)
        nat[:N_LOC] = t16[sl]
        # group-contiguous transposed layout: per group g, [128 d-part, 24 k, W n]
        tTg = np.empty((12, 128, KD, 512), dtype=np.float16)
        for g in range(12):
            blk = nat[g * 512:(g + 1) * 512, :]          # [512 n, 3072 d]
            tTg[g] = blk.reshape(512, KD, 128).transpose(2, 1, 0)
        tTl = nat[6144:6272, :].reshape(128, KD, 128).transpose(2, 1, 0)
        trsq_c = np.zeros((N_PAD,), dtype=np.float32)
        trsq_c[:N_LOC] = trsq_full[sl]
        hi = trsq_c.astype(np.float16)
        lo = (trsq_c.astype(np.float64) - hi.astype(np.float64)).astype(np.float16)
        trsqg = np.zeros((96, 5, 512), dtype=np.float16)
        for g in range(NG):
            w = GROUPS[g][1]
            trsqg[32 * (g % 3), g // 3, :w] = hi[g * 512:g * 512 + w]
            trsqg[32 * (g % 3) + 1, g // 3, :w] = lo[g * 512:g * 512 + w]
        in_maps.append({
            "tTg": np.ascontiguousarray(tTg.reshape(12 * 128, KD * 512)),
            "tTl": np.ascontiguousarray(tTl.reshape(128, KD * 128)),
            "tn": nat,
            "xT": x16T,
            "ident": ident,
            "trsq": trsqg.reshape(96, 5 * 512),
            "negs": negs,
            "gcol": g_col,
        })

    nc = _get_nc()
    res = run_bass_kernel_spmd(nc, in_maps, list(range(NCORES)))
    _CACHED["last_results"] = res

    # --- host-side online-softmax merge across cores (fp64) ---
    g64 = gp32.astype(np.float64)
    Ms = np.stack([res.results[c]["m_out"][:, 0].astype(np.float64)
                   for c in range(NCORES)])          # [C, B]
    Ss = np.stack([res.results[c]["s_out"][:, 0].astype(np.float64)
                   for c in range(NCORES)])          # [C, B]
    ACCs = np.stack([res.results[c]["acc_out"].astype(np.float64)
                     for c in range(NCORES)])        # [C, B, D]
    Mg = Ms.max(axis=0)                              # [B]
    scale = np.exp(g64[None, :] * (Ms - Mg[None, :]))  # [C, B]
    den = (scale * Ss).sum(axis=0)                   # [B]
    num = (scale[:, :, None] * ACCs).sum(axis=0)     # [B, D]
    weighted = num / den[:, None]

    coef_x = 1.0 / np.sqrt(om)
    coef_x_hat = a / np.sqrt(om)
    out = coef_x[:, None] * xf.astype(np.float64) - coef_x_hat[:, None] * weighted
    return out.reshape(x.shape).astype(np.float32)
